# revision 26
# baseline (speedup 1.0000x reference)
"""Trainium2 Bass kernel for ApplyBiasRopeUpdateKVCache (8-core SPMD).

Problem (hardcoded shapes):
  qkv [16384, 6144] f32  = [T, (32 q + 8 k + 8 v heads) * 128]
  block_tables [4, 64] int32, kv_scale_orig_quant [1] f32
  -> qkv_out [16384, 6144] f32 (NeoX RoPE on q/k, v passthrough)
     k_cache, v_cache [256, 64, 8, 128] int8 (paged scatter of int8-quantized
     rope'd k and raw v)

Sharding: by KV head across 8 cores. Core d gets q heads 4d..4d+4, kv head d
(columns re-packed host-side into one contiguous [16384, 768] slice), and owns
cache partition [:, :, d, :].

Layout: token-quad packing. A tile covers 512 tokens as [128 partitions,
4 chunks x 768 cols]; partition p chunk j holds token 512*i + 4*p + j, so
every DMA descriptor covers 4 consecutive DRAM rows: 12 KB for x/y, 512 B
for the int8 caches (vs 3 KB / 128 B with block-chunk packing). cos/sin
tables are host-permuted into the same (partition, class, chunk) layout.

Device kernel per tile:
  A (x4): tmp = swap_halves(x) * sin_signed  (DVE, negative-stride AP)
  C: x[rope cols] *= cos_rep (broadcast over heads, in-place)
  D: x[rope cols] += tmp                     (x is now [rope(q)|rope(k)|v])
  kq/vq = int8(round(k_rot*scale)), int8(round(v*scale)) on the ACT engine
  (hw f32->int8 cast is round-nearest-even + saturate = jnp round+clip)
  DMA x -> qkv_out rows, kq/vq -> cache rows at block_tables destinations
  (block table values are baked into DMA descriptors at build time).

Queue discipline: Sync queue is a pure input-prefetch stream; everything
gated on a tile's DVE output (y-out, quants, cache stores) issues from the
Scalar queue. No gpsimd DMAs (SWDGE would be starved by DVE holding the
shared SBUF port).
"""
import numpy as np
from contextlib import ExitStack

import concourse.bass as bass
import concourse.tile as tile
from concourse import bacc, mybir
from concourse.alu_op_type import AluOpType
from concourse.bass_utils import run_bass_kernel_spmd

H, HKV, D = 32, 8, 128
B, S = 4, 4096
T = B * S                 # 16384 tokens
TPB = 64                  # tokens per cache block
NUM_BLOCKS = B * S // TPB # 256
HALF = D // 2             # 64
THETA = 10000.0
SCALING = 1.0
NCORES = 8
QH = H // NCORES          # 4 q heads per core
NH = QH + 1               # rope'd heads per core (q + k)
ROT_W = NH * D            # 640
QW = QH * D               # 512 q cols per core
W = (QH + 2) * D          # 768 cols per core (q|k|v)
PT = 128                  # partitions
CHUNKS = 4                # tokens per partition (consecutive in DRAM)
TT = PT * CHUNKS          # 512 tokens per tile
NT = T // TT              # 32 tiles
NCLS = S // TT            # 8 distinct table classes (tile position patterns)

_prog_cache: dict = {}


def _build_program(scale: float, dest_rows: tuple):
    """Build + compile the SPMD program. dest_rows[g] is the flat cache row
    (block*TPB) for token group g (tokens 64g..64g+64)."""
    key = (scale, dest_rows)
    if key in _prog_cache:
        return _prog_cache[key]

    nc = bacc.Bacc("TRN2", target_bir_lowering=False, debug=False)
    f32 = mybir.dt.float32
    i8 = mybir.dt.int8

    x_ext = nc.dram_tensor("x", [T, W], f32, kind="ExternalInput").ap()
    # tables ship half-width (64 per chunk); device expands to 128-wide
    # [cos|cos] and [-sin|sin] with exact copies / sign flips
    cos_ext = nc.dram_tensor("costab", [PT, NCLS * CHUNKS * HALF], f32,
                             kind="ExternalInput").ap()
    sin_ext = nc.dram_tensor("sintab", [PT, NCLS * CHUNKS * HALF], f32,
                             kind="ExternalInput").ap()
    y_ext = nc.dram_tensor("y", [T, W], f32, kind="ExternalOutput").ap()
    kc_ext = nc.dram_tensor("kc", [NUM_BLOCKS * TPB, D], i8, kind="ExternalOutput").ap()
    vc_ext = nc.dram_tensor("vc", [NUM_BLOCKS * TPB, D], i8, kind="ExternalOutput").ap()

    with tile.TileContext(nc) as tc:
        with ExitStack() as ctx:
            cpool = ctx.enter_context(tc.tile_pool(name="consts", bufs=1))
            xpool = ctx.enter_context(tc.tile_pool(name="x", bufs=6))
            qbpool = ctx.enter_context(tc.tile_pool(name="qb", bufs=4))
            tqpool = ctx.enter_context(tc.tile_pool(name="tmq", bufs=4))
            tkpool = ctx.enter_context(tc.tile_pool(name="tmk", bufs=4))
            qpool = ctx.enter_context(tc.tile_pool(name="q8", bufs=8))

            # resident cos/sin tables laid out [p, class*CHUNKS*D]. The DMA
            # ships the 64-wide halves; one copy + one sign-flip per table
            # expand them to [cos|cos] and [-sin|sin] (both bit-exact). sin
            # loads first on the input (sync) queue — op A needs it before
            # anything else; cos rides the idle scalar queue so the
            # x-prefetch stream starts immediately after sin. The q path
            # computes in bf16 (DVE 2x mode), so bf16 copies of both tables
            # are cast once on device.
            bf16 = mybir.dt.bfloat16
            cost = cpool.tile([PT, NCLS * CHUNKS * D], f32)
            sint = cpool.tile([PT, NCLS * CHUNKS * D], f32)
            costb = cpool.tile([PT, NCLS * CHUNKS * D], bf16)
            sintb = cpool.tile([PT, NCLS * CHUNKS * D], bf16)
            half_in = cpool.tile([PT, NCLS * CHUNKS * HALF], f32, tag="half_s")
            half_ic = cpool.tile([PT, NCLS * CHUNKS * HALF], f32, tag="half_c")
            nc.sync.dma_start(out=half_in[:], in_=sin_ext[:])
            nc.scalar.dma_start(out=half_ic[:], in_=cos_ext[:])
            for src, dst, dstb, neg_lo in ((half_in, sint, sintb, True),
                                           (half_ic, cost, costb, False)):
                s3 = src[:].rearrange("p (q d) -> p q d", d=HALF)
                d4 = dst[:].rearrange("p (q s d) -> p q s d", s=2, d=HALF)
                nc.vector.tensor_scalar(
                    out=d4[:, :, 0, :], in0=s3,
                    scalar1=-1.0 if neg_lo else 1.0, scalar2=None,
                    op0=AluOpType.mult)
                nc.vector.tensor_copy(out=d4[:, :, 1, :], in_=s3)
                nc.vector.tensor_copy(out=dstb[:], in_=dst[:])

            for i in range(NT):
                m = i % NCLS  # table class
                xt = xpool.tile([PT, CHUNKS * W], f32)
                nc.sync.dma_start(
                    out=xt[:].rearrange("p (j c) -> p j c", j=CHUNKS),
                    in_=x_ext[i * TT:(i + 1) * TT, :].rearrange(
                        "(p j) c -> p j c", j=CHUNKS))

                x3 = xt[:].rearrange("p (j c) -> p j c", j=CHUNKS)
                mo = m * CHUNKS * D

                # ---- Q path in bf16 (DVE tensor_tensor runs 2x for 16-bit;
                # fp32 is hardwired to 1x). Q only feeds qkv_out (2e-2 rel
                # err gate); K stays f32 so the int8 caches stay bit-exact.
                qb = qbpool.tile([PT, CHUNKS * QW], bf16)
                qb3 = qb[:].rearrange("p (j c) -> p j c", j=CHUNKS)
                nc.scalar.activation(out=qb3, in_=x3[:, :, 0:QW],
                                     func=mybir.ActivationFunctionType.Copy)

                tmq = tqpool.tile([PT, CHUNKS * QW], bf16)
                for j in range(CHUNKS):
                    sin_j = sintb[:, mo + j * D:mo + (j + 1) * D]
                    tmq4 = tmq[:, j * QW:(j + 1) * QW].rearrange(
                        "p (h s d) -> p h s d", h=QH, s=2)
                    q_sw = qb[:, j * QW:(j + 1) * QW].rearrange(
                        "p (h s d) -> p h s d", h=QH, s=2)[:, :, ::-1, :]
                    sin4 = (sin_j.rearrange("p (s d) -> p s d", s=2)
                            .unsqueeze(1).broadcast_to([PT, QH, 2, HALF]))
                    nc.vector.tensor_tensor(out=tmq4, in0=q_sw, in1=sin4,
                                            op=AluOpType.mult)
                qb4 = qb3.rearrange("p j (h c) -> p j h c", h=QH)
                cosq = (costb[:, mo:mo + CHUNKS * D]
                        .rearrange("p (j c) -> p j c", j=CHUNKS)
                        .unsqueeze(2).broadcast_to([PT, CHUNKS, QH, D]))
                nc.vector.tensor_tensor(out=qb4, in0=qb4, in1=cosq,
                                        op=AluOpType.mult)
                tmq3 = tmq[:].rearrange("p (j c) -> p j c", j=CHUNKS)
                nc.vector.tensor_tensor(out=qb3, in0=qb3, in1=tmq3,
                                        op=AluOpType.add)
                # rope'd q back into xt (f32) for the single y-out DMA
                nc.scalar.activation(out=x3[:, :, 0:QW], in_=qb3,
                                     func=mybir.ActivationFunctionType.Copy)

                # ---- K path in f32 (bit-exact vs the jax reference)
                tmk = tkpool.tile([PT, CHUNKS * D], f32)
                for j in range(CHUNKS):
                    sin_j = sint[:, mo + j * D:mo + (j + 1) * D]
                    tmk3 = tmk[:, j * D:(j + 1) * D].rearrange(
                        "p (s d) -> p s d", s=2)
                    k_sw = xt[:, j * W + QW:j * W + ROT_W].rearrange(
                        "p (s d) -> p s d", s=2)[:, ::-1, :]
                    sin3 = sin_j.rearrange("p (s d) -> p s d", s=2)
                    nc.vector.tensor_tensor(out=tmk3, in0=k_sw, in1=sin3,
                                            op=AluOpType.mult)
                xk = x3[:, :, QW:ROT_W]
                cosk = (cost[:, mo:mo + CHUNKS * D]
                        .rearrange("p (j c) -> p j c", j=CHUNKS))
                nc.vector.tensor_tensor(out=xk, in0=xk, in1=cosk,
                                        op=AluOpType.mult)
                tmk3f = tmk[:].rearrange("p (j c) -> p j c", j=CHUNKS)
                nc.vector.tensor_tensor(out=xk, in0=xk, in1=tmk3f,
                                        op=AluOpType.add)

                # quantize k_rot and v to int8 (RNE + saturate in the cast)
                kq = qpool.tile([PT, CHUNKS * D], i8, tag="kq")
                vq = qpool.tile([PT, CHUNKS * D], i8, tag="vq")
                k3 = kq[:].rearrange("p (j c) -> p j c", j=CHUNKS)
                v3 = vq[:].rearrange("p (j c) -> p j c", j=CHUNKS)
                nc.scalar.activation(out=k3, in_=x3[:, :, QH * D:ROT_W],
                                     func=mybir.ActivationFunctionType.Copy,
                                     scale=scale)
                nc.scalar.activation(out=v3, in_=x3[:, :, ROT_W:W],
                                     func=mybir.ActivationFunctionType.Copy,
                                     scale=scale)

                # qkv_out rows (issued from the Scalar sequencer)
                nc.scalar.dma_start(
                    out=y_ext[i * TT:(i + 1) * TT, :].rearrange(
                        "(p j) c -> p j c", j=CHUNKS),
                    in_=xt[:].rearrange("p (j c) -> p j c", j=CHUNKS))

                # cache rows: one DMA per tensor when the tile's 8 dest
                # blocks are consecutive (common case), else per 64-token
                # group (16 partitions each)
                g0 = (TT // TPB) * i
                runs_contig = all(
                    dest_rows[g0 + g + 1] == dest_rows[g0 + g] + TPB
                    for g in range(TT // TPB - 1))
                for src, cache in ((kq, kc_ext), (vq, vc_ext)):
                    if runs_contig:
                        r0 = dest_rows[g0]
                        nc.scalar.dma_start(
                            out=cache[r0:r0 + TT, :].rearrange(
                                "(p j) c -> p j c", j=CHUNKS),
                            in_=src[:].rearrange("p (j c) -> p j c", j=CHUNKS))
                    else:
                        ppg = TPB // CHUNKS  # partitions per 64-token group
                        for g in range(TT // TPB):
                            r = dest_rows[g0 + g]
                            nc.scalar.dma_start(
                                out=cache[r:r + TPB, :].rearrange(
                                    "(p j) c -> p j c", j=CHUNKS),
                                in_=src[g * ppg:(g + 1) * ppg, :].rearrange(
                                    "p (j c) -> p j c", j=CHUNKS))

    nc.compile()
    _prog_cache[key] = nc
    return nc


def _tables():
    """cos/sin tables, bit-identical to the jax-CPU reference computation,
    permuted into the device layout [p, class*CHUNKS*D] where entry
    (p, m, j, c) = table[512*m + 4*p + j, c]."""
    try:
        import jax
        import jax.numpy as jnp
        with jax.default_device(jax.devices("cpu")[0]):
            pos = jnp.arange(S, dtype=jnp.int32).astype(jnp.float32) / SCALING
            inv_freq = 1.0 / (THETA ** (jnp.arange(HALF, dtype=jnp.float32)
                                        / HALF))
            ang = pos[:, None] * inv_freq[None, :]
            c = np.asarray(jnp.cos(ang), dtype=np.float32)
            s = np.asarray(jnp.sin(ang), dtype=np.float32)
    except Exception:
        pos = np.arange(S, dtype=np.float32)
        inv_freq = (1.0 / (THETA ** (np.arange(HALF, dtype=np.float64)
                                     / HALF))).astype(np.float32)
        ang = pos[:, None].astype(np.float64) * inv_freq[None, :]
        c = np.cos(ang).astype(np.float32)
        s = np.sin(ang).astype(np.float32)
    p = np.arange(PT)
    m = np.arange(NCLS)
    j = np.arange(CHUNKS)
    idx = (TT * m[None, :, None] + CHUNKS * p[:, None, None]
           + j[None, None, :])                        # [PT, NCLS, CHUNKS]
    costab = c[idx].reshape(PT, NCLS * CHUNKS * HALF)
    sintab = s[idx].reshape(PT, NCLS * CHUNKS * HALF)
    return np.ascontiguousarray(costab), np.ascontiguousarray(sintab)


def kernel(qkv, block_tables, kv_scale_orig_quant, _trace=False):
    qkv = np.ascontiguousarray(np.asarray(qkv), dtype=np.float32)
    block_tables = np.asarray(block_tables)
    scale = float(np.asarray(kv_scale_orig_quant).reshape(-1)[0])

    dest_rows = tuple(int(block_tables[g // (S // TPB), g % (S // TPB)]) * TPB
                      for g in range(NUM_BLOCKS))
    nc = _build_program(scale, dest_rows)

    costab, sintab = _tables()
    x = qkv.reshape(T, H + 2 * HKV, D)
    in_maps = []
    for d in range(NCORES):
        sl = np.concatenate(
            [x[:, 4 * d:4 * d + 4, :].reshape(T, QH * D),
             x[:, H + d, :], x[:, H + HKV + d, :]], axis=1)
        in_maps.append({"x": np.ascontiguousarray(sl),
                        "costab": costab, "sintab": sintab})

    res = run_bass_kernel_spmd(nc, in_maps, core_ids=list(range(NCORES)),
                               trace=_trace)

    qkv_out = np.empty((T, (H + 2 * HKV) * D), dtype=np.float32)
    qo = qkv_out.reshape(T, H + 2 * HKV, D)
    k_cache = np.empty((NUM_BLOCKS, TPB, HKV, D), dtype=np.int8)
    v_cache = np.empty((NUM_BLOCKS, TPB, HKV, D), dtype=np.int8)
    for d in range(NCORES):
        y = res.results[d]["y"]
        qo[:, 4 * d:4 * d + 4, :] = y[:, :QH * D].reshape(T, QH, D)
        qo[:, H + d, :] = y[:, QH * D:ROT_W]
        qo[:, H + HKV + d, :] = y[:, ROT_W:W]
        k_cache[:, :, d, :] = res.results[d]["kc"].reshape(NUM_BLOCKS, TPB, D)
        v_cache[:, :, d, :] = res.results[d]["vc"].reshape(NUM_BLOCKS, TPB, D)

    if _trace:
        kernel.last_exec_time_ns = res.exec_time_ns
    return qkv_out, k_cache, v_cache


kernel.last_exec_time_ns = None


# revision 27
# speedup vs baseline: 1.3449x; 1.3449x over previous
"""Trainium2 Bass kernel for ApplyBiasRopeUpdateKVCache (8-core SPMD).

Problem (hardcoded shapes):
  qkv [16384, 6144] f32  = [T, (32 q + 8 k + 8 v heads) * 128]
  block_tables [4, 64] int32, kv_scale_orig_quant [1] f32
  -> qkv_out [16384, 6144] f32 (NeoX RoPE on q/k, v passthrough)
     k_cache, v_cache [256, 64, 8, 128] int8 (paged scatter of int8-quantized
     rope'd k and raw v)

Sharding: by KV head across 8 cores. Core d gets q heads 4d..4d+4, kv head d
(columns re-packed host-side into one contiguous [16384, 768] slice), and owns
cache partition [:, :, d, :].

Layout: token-quad packing. A tile covers 512 tokens as [128 partitions,
4 chunks x 768 cols]; partition p chunk j holds token 512*i + 4*p + j, so
every DMA descriptor covers 4 consecutive DRAM rows: 12 KB for x/y, 512 B
for the int8 caches (vs 3 KB / 128 B with block-chunk packing). cos/sin
tables are host-permuted into the same (partition, class, chunk) layout.

Device kernel per tile:
  A (x4): tmp = swap_halves(x) * sin_signed  (DVE, negative-stride AP)
  C: x[rope cols] *= cos_rep (broadcast over heads, in-place)
  D: x[rope cols] += tmp                     (x is now [rope(q)|rope(k)|v])
  kq/vq = int8(round(k_rot*scale)), int8(round(v*scale)) on the ACT engine
  (hw f32->int8 cast is round-nearest-even + saturate = jnp round+clip)
  DMA x -> qkv_out rows, kq/vq -> cache rows at block_tables destinations
  (block table values are baked into DMA descriptors at build time).

Queue discipline: Sync queue is a pure input-prefetch stream; everything
gated on a tile's DVE output (y-out, quants, cache stores) issues from the
Scalar queue. No gpsimd DMAs (SWDGE would be starved by DVE holding the
shared SBUF port).
"""
import numpy as np
from contextlib import ExitStack

import concourse.bass as bass
import concourse.tile as tile
from concourse import bacc, mybir
from concourse.alu_op_type import AluOpType
from concourse.bass_utils import run_bass_kernel_spmd

H, HKV, D = 32, 8, 128
B, S = 4, 4096
T = B * S                 # 16384 tokens
TPB = 64                  # tokens per cache block
NUM_BLOCKS = B * S // TPB # 256
HALF = D // 2             # 64
THETA = 10000.0
SCALING = 1.0
NCORES = 8
QH = H // NCORES          # 4 q heads per core
NH = QH + 1               # rope'd heads per core (q + k)
ROT_W = NH * D            # 640
W = (QH + 2) * D          # 768 cols per core (q|k|v)
PT = 128                  # partitions
CHUNKS = 4                # tokens per partition (consecutive in DRAM)
TT = PT * CHUNKS          # 512 tokens per tile
NT = T // TT              # 32 tiles
NCLS = S // TT            # 8 distinct table classes (tile position patterns)

_prog_cache: dict = {}


def _build_program(scale: float, dest_rows: tuple):
    """Build + compile the SPMD program. dest_rows[g] is the flat cache row
    (block*TPB) for token group g (tokens 64g..64g+64)."""
    key = (scale, dest_rows)
    if key in _prog_cache:
        return _prog_cache[key]

    nc = bacc.Bacc("TRN2", target_bir_lowering=False, debug=False)
    f32 = mybir.dt.float32
    i8 = mybir.dt.int8

    x_ext = nc.dram_tensor("x", [T, W], f32, kind="ExternalInput").ap()
    # tables ship half-width (64 per chunk); device expands to 128-wide
    # [cos|cos] and [-sin|sin] with exact copies / sign flips
    cos_ext = nc.dram_tensor("costab", [PT, NCLS * CHUNKS * HALF], f32,
                             kind="ExternalInput").ap()
    sin_ext = nc.dram_tensor("sintab", [PT, NCLS * CHUNKS * HALF], f32,
                             kind="ExternalInput").ap()
    y_ext = nc.dram_tensor("y", [T, W], f32, kind="ExternalOutput").ap()
    kc_ext = nc.dram_tensor("kc", [NUM_BLOCKS * TPB, D], i8, kind="ExternalOutput").ap()
    vc_ext = nc.dram_tensor("vc", [NUM_BLOCKS * TPB, D], i8, kind="ExternalOutput").ap()

    with tile.TileContext(nc) as tc:
        with ExitStack() as ctx:
            cpool = ctx.enter_context(tc.tile_pool(name="consts", bufs=1))
            xpool = ctx.enter_context(tc.tile_pool(name="x", bufs=7))
            tpool = ctx.enter_context(tc.tile_pool(name="tmp", bufs=4))
            qpool = ctx.enter_context(tc.tile_pool(name="q8", bufs=8))

            # resident cos/sin tables laid out [p, class*CHUNKS*D]. The DMA
            # ships the 64-wide halves; one copy + one sign-flip per table
            # expand them to [cos|cos] and [-sin|sin] (both bit-exact). sin
            # loads first on the input (sync) queue — op A needs it before
            # anything else; cos rides the idle scalar queue so the
            # x-prefetch stream starts immediately after sin.
            cost = cpool.tile([PT, NCLS * CHUNKS * D], f32)
            sint = cpool.tile([PT, NCLS * CHUNKS * D], f32)
            half_in = cpool.tile([PT, NCLS * CHUNKS * HALF], f32, tag="half_s")
            half_ic = cpool.tile([PT, NCLS * CHUNKS * HALF], f32, tag="half_c")
            nc.sync.dma_start(out=half_in[:], in_=sin_ext[:])
            nc.scalar.dma_start(out=half_ic[:], in_=cos_ext[:])
            for src, dst, neg_lo in ((half_in, sint, True),
                                     (half_ic, cost, False)):
                s3 = src[:].rearrange("p (q d) -> p q d", d=HALF)
                d4 = dst[:].rearrange("p (q s d) -> p q s d", s=2, d=HALF)
                nc.vector.tensor_scalar(
                    out=d4[:, :, 0, :], in0=s3,
                    scalar1=-1.0 if neg_lo else 1.0, scalar2=None,
                    op0=AluOpType.mult)
                nc.vector.tensor_copy(out=d4[:, :, 1, :], in_=s3)

            for i in range(NT):
                m = i % NCLS  # table class
                xt = xpool.tile([PT, CHUNKS * W], f32)
                nc.sync.dma_start(
                    out=xt[:].rearrange("p (j c) -> p j c", j=CHUNKS),
                    in_=x_ext[i * TT:(i + 1) * TT, :].rearrange(
                        "(p j) c -> p j c", j=CHUNKS))

                # A: tmp[p,j,h,s,d] = x[p,j,h,1-s,d] * sin_signed[p,m,j,s,d]
                # (per chunk j: walrus lowering caps APs at 4 dims)
                tmp = tpool.tile([PT, CHUNKS * ROT_W], f32)
                for j in range(CHUNKS):
                    sin_j = sint[:, (m * CHUNKS + j) * D:(m * CHUNKS + j + 1) * D]
                    tmp4 = tmp[:, j * ROT_W:(j + 1) * ROT_W].rearrange(
                        "p (h s d) -> p h s d", h=NH, s=2)
                    x_sw = xt[:, j * W:j * W + ROT_W].rearrange(
                        "p (h s d) -> p h s d", h=NH, s=2)[:, :, ::-1, :]
                    sin4 = (sin_j.rearrange("p (s d) -> p s d", s=2)
                            .unsqueeze(1).broadcast_to([PT, NH, 2, HALF]))
                    nc.vector.tensor_tensor(out=tmp4, in0=x_sw, in1=sin4,
                                            op=AluOpType.mult)

                # C: x[rope] *= cos (broadcast over heads), all chunks at once
                x4 = xt[:].rearrange("p (j c) -> p j c", j=CHUNKS)[
                    :, :, :ROT_W].rearrange("p j (h c) -> p j h c", h=NH)
                cos4 = (cost[:, m * CHUNKS * D:(m + 1) * CHUNKS * D]
                        .rearrange("p (j c) -> p j c", j=CHUNKS)
                        .unsqueeze(2).broadcast_to([PT, CHUNKS, NH, D]))
                nc.vector.tensor_tensor(out=x4, in0=x4, in1=cos4,
                                        op=AluOpType.mult)

                # D: x[rope] += tmp, all chunks at once
                xr = xt[:].rearrange("p (j c) -> p j c", j=CHUNKS)[:, :, :ROT_W]
                tr = tmp[:].rearrange("p (j c) -> p j c", j=CHUNKS)
                nc.vector.tensor_tensor(out=xr, in0=xr, in1=tr,
                                        op=AluOpType.add)

                # quantize k_rot and v to int8 (RNE + saturate in the cast)
                kq = qpool.tile([PT, CHUNKS * D], i8, tag="kq")
                vq = qpool.tile([PT, CHUNKS * D], i8, tag="vq")
                x3 = xt[:].rearrange("p (j c) -> p j c", j=CHUNKS)
                k3 = kq[:].rearrange("p (j c) -> p j c", j=CHUNKS)
                v3 = vq[:].rearrange("p (j c) -> p j c", j=CHUNKS)
                nc.scalar.activation(out=k3, in_=x3[:, :, QH * D:ROT_W],
                                     func=mybir.ActivationFunctionType.Copy,
                                     scale=scale)
                nc.scalar.activation(out=v3, in_=x3[:, :, ROT_W:W],
                                     func=mybir.ActivationFunctionType.Copy,
                                     scale=scale)

                # qkv_out rows (issued from the Scalar sequencer)
                nc.scalar.dma_start(
                    out=y_ext[i * TT:(i + 1) * TT, :].rearrange(
                        "(p j) c -> p j c", j=CHUNKS),
                    in_=xt[:].rearrange("p (j c) -> p j c", j=CHUNKS))

                # cache rows: one DMA per tensor when the tile's 8 dest
                # blocks are consecutive (common case), else per 64-token
                # group (16 partitions each)
                g0 = (TT // TPB) * i
                runs_contig = all(
                    dest_rows[g0 + g + 1] == dest_rows[g0 + g] + TPB
                    for g in range(TT // TPB - 1))
                for src, cache in ((kq, kc_ext), (vq, vc_ext)):
                    if runs_contig:
                        r0 = dest_rows[g0]
                        nc.scalar.dma_start(
                            out=cache[r0:r0 + TT, :].rearrange(
                                "(p j) c -> p j c", j=CHUNKS),
                            in_=src[:].rearrange("p (j c) -> p j c", j=CHUNKS))
                    else:
                        ppg = TPB // CHUNKS  # partitions per 64-token group
                        for g in range(TT // TPB):
                            r = dest_rows[g0 + g]
                            nc.scalar.dma_start(
                                out=cache[r:r + TPB, :].rearrange(
                                    "(p j) c -> p j c", j=CHUNKS),
                                in_=src[g * ppg:(g + 1) * ppg, :].rearrange(
                                    "p (j c) -> p j c", j=CHUNKS))

    nc.compile()
    _prog_cache[key] = nc
    return nc


def _tables():
    """cos/sin tables, bit-identical to the jax-CPU reference computation,
    permuted into the device layout [p, class*CHUNKS*D] where entry
    (p, m, j, c) = table[512*m + 4*p + j, c]."""
    try:
        import jax
        import jax.numpy as jnp
        with jax.default_device(jax.devices("cpu")[0]):
            pos = jnp.arange(S, dtype=jnp.int32).astype(jnp.float32) / SCALING
            inv_freq = 1.0 / (THETA ** (jnp.arange(HALF, dtype=jnp.float32)
                                        / HALF))
            ang = pos[:, None] * inv_freq[None, :]
            c = np.asarray(jnp.cos(ang), dtype=np.float32)
            s = np.asarray(jnp.sin(ang), dtype=np.float32)
    except Exception:
        pos = np.arange(S, dtype=np.float32)
        inv_freq = (1.0 / (THETA ** (np.arange(HALF, dtype=np.float64)
                                     / HALF))).astype(np.float32)
        ang = pos[:, None].astype(np.float64) * inv_freq[None, :]
        c = np.cos(ang).astype(np.float32)
        s = np.sin(ang).astype(np.float32)
    p = np.arange(PT)
    m = np.arange(NCLS)
    j = np.arange(CHUNKS)
    idx = (TT * m[None, :, None] + CHUNKS * p[:, None, None]
           + j[None, None, :])                        # [PT, NCLS, CHUNKS]
    costab = c[idx].reshape(PT, NCLS * CHUNKS * HALF)
    sintab = s[idx].reshape(PT, NCLS * CHUNKS * HALF)
    return np.ascontiguousarray(costab), np.ascontiguousarray(sintab)


def kernel(qkv, block_tables, kv_scale_orig_quant, _trace=False):
    qkv = np.ascontiguousarray(np.asarray(qkv), dtype=np.float32)
    block_tables = np.asarray(block_tables)
    scale = float(np.asarray(kv_scale_orig_quant).reshape(-1)[0])

    dest_rows = tuple(int(block_tables[g // (S // TPB), g % (S // TPB)]) * TPB
                      for g in range(NUM_BLOCKS))
    nc = _build_program(scale, dest_rows)

    costab, sintab = _tables()
    x = qkv.reshape(T, H + 2 * HKV, D)
    in_maps = []
    for d in range(NCORES):
        sl = np.concatenate(
            [x[:, 4 * d:4 * d + 4, :].reshape(T, QH * D),
             x[:, H + d, :], x[:, H + HKV + d, :]], axis=1)
        in_maps.append({"x": np.ascontiguousarray(sl),
                        "costab": costab, "sintab": sintab})

    res = run_bass_kernel_spmd(nc, in_maps, core_ids=list(range(NCORES)),
                               trace=_trace)

    qkv_out = np.empty((T, (H + 2 * HKV) * D), dtype=np.float32)
    qo = qkv_out.reshape(T, H + 2 * HKV, D)
    k_cache = np.empty((NUM_BLOCKS, TPB, HKV, D), dtype=np.int8)
    v_cache = np.empty((NUM_BLOCKS, TPB, HKV, D), dtype=np.int8)
    for d in range(NCORES):
        y = res.results[d]["y"]
        qo[:, 4 * d:4 * d + 4, :] = y[:, :QH * D].reshape(T, QH, D)
        qo[:, H + d, :] = y[:, QH * D:ROT_W]
        qo[:, H + HKV + d, :] = y[:, ROT_W:W]
        k_cache[:, :, d, :] = res.results[d]["kc"].reshape(NUM_BLOCKS, TPB, D)
        v_cache[:, :, d, :] = res.results[d]["vc"].reshape(NUM_BLOCKS, TPB, D)

    if _trace:
        kernel.last_exec_time_ns = res.exec_time_ns
    return qkv_out, k_cache, v_cache


kernel.last_exec_time_ns = None
